# revision 1
# baseline (speedup 1.0000x reference)
"""Trainium2 Bass kernel for nn_AttentionBlock (GroupNorm + single attn block + proj).

Sharding: the spatial axis t = H*W = 4096 is split across 8 cores (512 columns
each).  GroupNorm and the k/v projections are replicated on every core (they
need the full sequence); q, the attention scores, softmax, AV, the output
projection and the residual are computed only for the core's own t-columns,
so the gather is a pure concat along t.

Device algorithm per core:
  - The ENTIRE GroupNorm fold happens on the host in fp32: exact per-group
    mean/var, A = gamma*istd scaled into the qkv weight columns, and the
    B = beta - mean*A offset folded into the q bias (exactly) and into b_p
    via v's bias; k's offset is constant along the softmax axis and
    cancels.  The host ships x and the folded qkv weights as fp8e4 in
    DoubleRow pair layout [128, 2, *], plus an f32 x-chunk for the
    residual.  The device preamble is just DMAs + the q projection.
  - q/k/v are single fp8 DoubleRow matmuls per tile (K=256 contracted as
    2x128 at 0.5 cycles/row).  q gets SCALE2 (both attention scales) and
    bq via host fold + ACT conversion; k stays f32r for the S matmuls; vT
    is stored fp8e4 with an all-ones column per head so the AV matmul also
    emits the softmax denominator.
  - Attention stream, one (head, s-block-pair) slot at a time:
      * two S^T matmuls (f32r, K=32), each into its own 1-bank PSUM tile;
        the 4-deep score rotation decouples slot g+2's S matmuls from slot
        g's exp (a 2x2-bank rotation serializes the two exp-engine chains)
      * softmax exp on EITHER ScalarE (table exp -> fp8, logits shifted -2)
        OR DVE (Schraudolph: byte = S*8*log2(e) + const, one fused mult-add
        with saturating-to-[0,255] uint8 convert, bitcast to fp8e4).  Slots
        split 79/49 between the engines so both exp streams run
        concurrently -- exp throughput is the kernel's limit.
      * one fp8 DoubleRow AV matmul per slot contracts the 256 s-rows of
        the pair at 0.5 cycles/row (pair-dim steps kept 16B-aligned).
  - Head tails in pairs: per head, reciprocal of the denominator row on
    DVE, partition-broadcast on the otherwise-idle Pool engine, at = pav*rb
    in fp8; per head-PAIR one fp8 DoubleRow projection matmul accumulates
    both heads, halving the PSUM-read adds into hout.  Last head runs one
    full-width chain at the very end.
"""

import math
from contextlib import ExitStack

import numpy as np

import concourse.bacc as bacc
import concourse.bass as bass
import concourse.mybir as mybir
import concourse.tile as tile

F32 = mybir.dt.float32
F32R = mybir.dt.float32r
F8 = mybir.dt.float8e4
BF16 = mybir.dt.bfloat16
U8 = mybir.dt.uint8
PM = mybir.MatmulPerfMode
AF = mybir.ActivationFunctionType
ALU = mybir.AluOpType
AX = mybir.AxisListType

C = 256           # channels
T = 4096          # h*w
NH = 8            # heads
CHD = 32          # channels per head
NCORES = 8
TC = T // NCORES  # 512 t-columns per core
NSB = T // 128    # 32 s-blocks of 128
NPAIR = NSB // 2  # 16 s-block pairs per head
EPS = 1e-5
SCALE2 = 1.0 / math.sqrt(CHD)   # (1/ch^0.25)^2 — both attention scales
NSUB = T // 512

# Schraudolph exp -> fp8e4 byte domain: byte(e^x) ~= x*8*log2(e) + 56.
# -2*SCH_A shifts logits by -2 (matches the ACT path's Exp bias); -0.33
# centers the piecewise-linear mantissa approximation (ratio in [0.97,1.03]).
SCH_A = 8.0 / math.log(2.0)
SCH_B = 56.0 - 2.0 * SCH_A - 0.33

# cvec column indices (packed [128,1] constants)
GA0, GA1, BE0, BE1, BP0, BP1, BQ0, BQ1, BQ2 = range(9)

# slots whose exp runs on DVE (Schraudolph) instead of ScalarE.  Spread
# through the stream; early slots stay on ACT while DVE finishes the first
# v-block copies.
N_DVE_EXP = 49
DVE_START = 3


def _dve_slots():
    s = set()
    for i in range(N_DVE_EXP):
        s.add(DVE_START + int(round(i * (128 - DVE_START) / N_DVE_EXP)))
    # measured: +1 on this slot clears a 5ns stall against head-3's tail
    s.discard(59)
    s.add(60)
    return s


def build_nc():
    nc = bacc.Bacc(trn_type="TRN2")

    x_8 = nc.dram_tensor("x_8", [C, T], F8, kind="ExternalInput")
    x_c = nc.dram_tensor("x_c", [C, TC], F32, kind="ExternalInput")
    # cvec (64B pad) + w8 (2080B) + x8c (1024B) packed as one byte blob:
    # a single HWDGE descriptor generation instead of three serialized ones
    blob = nc.dram_tensor("blob", [128, 3168], U8, kind="ExternalInput")
    w_p32 = nc.dram_tensor("w_p32", [CHD, NH * C], F8, kind="ExternalInput")
    out = nc.dram_tensor("out", [C, TC], F32, kind="ExternalOutput")

    dve_exp_slots = _dve_slots()

    with tile.TileContext(nc) as tc, ExitStack() as ctx:
        big = ctx.enter_context(tc.tile_pool(name="big", bufs=3))      # x then k
        cst = ctx.enter_context(tc.tile_pool(name="cst", bufs=1))
        med = ctx.enter_context(tc.tile_pool(name="med", bufs=1))
        sm = ctx.enter_context(tc.tile_pool(name="sm", bufs=2))
        pex = ctx.enter_context(tc.tile_pool(name="pex", bufs=8))
        ps_s = ctx.enter_context(tc.tile_pool(name="ps_s", bufs=4, space="PSUM"))
        ps_m = ctx.enter_context(tc.tile_pool(name="ps_m", bufs=2, space="PSUM"))
        ps_a = ctx.enter_context(tc.tile_pool(name="ps_a", bufs=2, space="PSUM"))

        # ---- loads.  The entire GroupNorm fold happens on the host (exact
        # mean/var in fp32, gamma*istd folded into the fp8 qkv weights, the
        # mean/beta offset folded into the q and b_p biases), so the device
        # preamble is just these DMAs + the q projection. ----
        x8 = med.tile([128, 2, T], F8, tag="x8", name="x8")
        blob_sb = cst.tile([128, 3168], U8, tag="blob", name="blob")
        xct = [sm.tile([128, TC], F32, tag=f"xct{j}", bufs=1, name=f"xct{j}") for j in range(2)]
        cv_sb = blob_sb[:, 0:36].bitcast(F32)
        # padded 1032 -> 1040 row: DoubleRow pair step must be 16B-aligned
        W8P = 1040
        W8Q, W8K, W8V = 0, 384, 768
        w8 = blob_sb[:, 64:64 + 2 * W8P].bitcast(F8).rearrange(
            "p (i f) -> p i f", i=2)
        x8c = blob_sb[:, 2144:2144 + 2 * TC].bitcast(F8).rearrange(
            "p (i f) -> p i f", i=2)
        wp_sb = cst.tile([CHD, NH, C], F8, tag="wp", name="wp")
        # DMA order follows the dependency chain: the blob (everything q and
        # k production need except x) first, then the first 512 x-columns
        # (k tile-0 chunk 0 + v blocks 0-3), then the bulk of x
        x8r = x_8[:].rearrange("(i p) t -> p i t", i=2)
        nc.sync.dma_start(out=blob_sb, in_=blob[:])
        nc.sync.dma_start(out=x8[:, :, 0:512], in_=x8r[:, :, 0:512])
        for i in range(2):
            nc.sync.dma_start(out=x8[:, i, 512:T], in_=x8r[:, i, 512:T])
        for j in range(2):
            nc.sync.dma_start(out=xct[j], in_=x_c[128 * j:128 * (j + 1), :])
        nc.sync.dma_start(out=wp_sb, in_=w_p32[:].rearrange("c (h o) -> c h o", h=NH))
        nbias = cst.tile([128, 1], F32, tag="nbias", name="nbias")
        nc.vector.memset(nbias, -2.0)

        bp_sb = [cv_sb[:, BP0 + j:BP0 + j + 1] for j in range(2)]
        bq_sb = [cv_sb[:, BQ0 + j:BQ0 + j + 1] for j in range(3)]

        # ---- q (chunk only, 3 head-slot tiles, one DoubleRow matmul each) ----
        q_sb = [sm.tile([128, TC], F32R, tag=f"q{j}", bufs=1, name=f"q{j}") for j in range(3)]
        for o in range(3):
            pq = ps_m.tile([128, TC], F32, tag="ps_m", name="pq")
            nc.tensor.matmul(
                pq[:], w8[:, :, W8Q + 128 * o:W8Q + 128 * (o + 1)],
                x8c[:], start=True, stop=True, perf_mode=PM.DoubleRow)
            if o > 0:
                # tiles 1-2 don't gate the stream start; DVE is idle here
                nc.vector.tensor_scalar_add(
                    out=q_sb[o][:], in0=pq[:], scalar1=bq_sb[o])
            else:
                nc.scalar.activation(
                    out=q_sb[o][:], in_=pq[:], func=AF.Identity, bias=bq_sb[o])

        k_sb = [big.tile([128, T], F32R, tag="xk", name="xk") for _ in range(3)]
        # per-s-block row padded 264 -> 272 bytes: DoubleRow LdWeights
        # requires the pair-dim step to be a multiple of 16 bytes
        VROW = 272
        vt_sb = med.tile([128, NSB, VROW], F8, tag="vt", name="vt")
        # ones column per head, set once for all 32 s-blocks (Pool engine)
        onesv = cst.tile([128, NSB, NH], F8, tag="onesv", name="onesv")
        nc.gpsimd.memset(onesv, 1.0)
        nc.gpsimd.tensor_copy(
            out=vt_sb[:, :, 0:NH * 33].rearrange(
                "p s (h c) -> p s h c", c=33)[:, :, :, 32],
            in_=onesv[:])

        # PSUM->SBUF copies alternate between ACT and DVE to balance load
        # every 3rd PSUM->SBUF copy goes to ACT, the rest to DVE (keeps the
        # more-loaded exp engine, ACT, mostly free for the exp stream)
        copy_eng = [0]

        def psum_copy(out_ap, in_ap):
            copy_eng[0] = (copy_eng[0] + 1) % 3
            if copy_eng[0] == 0:
                nc.scalar.copy(out=out_ap, in_=in_ap)
            else:
                nc.vector.tensor_copy(out=out_ap, in_=in_ap)

        def emit_k_chunk(o, nchunk):
            cs = slice(512 * nchunk, 512 * (nchunk + 1))
            pk = ps_m.tile([128, 512], F32, tag="ps_m", name="pk")
            nc.tensor.matmul(
                pk[:], w8[:, :, W8K + 128 * o:W8K + 128 * (o + 1)],
                x8[:, :, cs], start=True, stop=True, perf_mode=PM.DoubleRow)
            # no k bias: q.bk is constant along the softmax axis, cancels
            psum_copy(k_sb[o][:, cs], pk[:])

        def emit_v_block(sb):
            pv = ps_m.tile([128, NH * 33], F32, tag="ps_m", name="pv")
            nc.tensor.matmul(
                pv[:], x8[:, :, 128 * sb:128 * (sb + 1)],
                w8[:, :, W8V:W8V + NH * 33],
                start=True, stop=True, perf_mode=PM.DoubleRow)
            psum_copy(
                vt_sb[:, sb, 0:NH * 33].rearrange(
                    "p (h c) -> p h c", c=33)[:, :, 0:32],
                pv[:].rearrange("p (h c) -> p h c", c=33)[:, :, 0:32])

        # only k tile 0's first two chunks + the first v block-pair must
        # precede head 0's stream; the rest trickles through early slots
        for nchunk in (0, 1):
            emit_k_chunk(0, nchunk)
        for sb in (0, 1):
            emit_v_block(sb)

        # heads 0 and 1 interleave pair-by-pair so v production spreads over
        # 32 slots; heads 2-7 run sequentially after
        slot_seq = []
        for p in range(NPAIR):
            slot_seq.append((0, p))
            slot_seq.append((1, p))
        for h in range(2, NH):
            for p in range(NPAIR):
                slot_seq.append((h, p))
        # production per global slot: v blocks through heads 0-1's slots,
        # k tile 0's tail chunks through the first slots, k tile 1 through
        # head 2, k tile 2 through heads 4-5
        prod_for = {}

        def addprod(g, unit):
            prod_for.setdefault(g, []).append(unit)

        for b in range(2, NSB):
            addprod(b - 2, ("v1", b))
        for c in range(2, NSUB):
            addprod(4 * c - 6, ("kc", (0, c)))
        for n in range(NSUB):
            addprod(32 + 2 * n, ("kc", (1, n)))
            addprod(64 + 3 * n, ("kc", (2, n)))

        # ---- hout accumulators ----
        hout = [sm.tile([128, TC], F32, tag=f"ho{j}", bufs=1, name=f"ho{j}") for j in range(2)]
        hout_inited = [False]

        def init_hout():
            if not hout_inited[0]:
                hout_inited[0] = True
                for o in range(2):
                    nc.gpsimd.tensor_scalar_add(
                        out=hout[o][:], in0=xct[o][:], scalar1=bp_sb[o])

        # ---- attention stream ----
        at2_cur = {}   # pair index -> at2 tile

        def get_at2(h):
            pair = h // 2
            if pair not in at2_cur:
                at2_cur[pair] = sm.tile([CHD, 2, TC], F8, tag="at2", bufs=2,
                                        name="at2")
            return at2_cur[pair]

        def emit_pair_proj(hodd, fs=None, last=False):
            """One fp8 DoubleRow proj matmul for heads (hodd-1, hodd)."""
            pair = hodd // 2
            at2 = at2_cur[pair]
            cols = fs if fs is not None else slice(0, TC)
            n = cols.stop - cols.start
            for o in range(2):
                pp = ps_m.tile([128, n], F32, tag="ps_m", name="pp")
                nc.tensor.matmul(
                    pp[:], wp_sb[:, hodd - 1:hodd + 1, 128 * o:128 * (o + 1)],
                    at2[:, :, cols], start=True, stop=True,
                    perf_mode=PM.DoubleRow)
                nc.vector.tensor_add(out=hout[o][:, cols],
                                     in0=hout[o][:, cols], in1=pp[:])
                if last:
                    eng = nc.sync if o == 0 else nc.scalar
                    eng.dma_start(out=out[128 * o:128 * (o + 1), cols],
                                  in_=hout[o][:, cols])

        def emit_head_tail(h, pav, last=False):
            at2 = get_at2(h)
            if last:
                # final head: pure end latency; on-chip broadcast via a tiny
                # ones-matmul, pipelined in column halves
                NQ = 1
                rbs = []
                for hf in range(NQ):
                    fs = slice(TC // NQ * hf, TC // NQ * (hf + 1))
                    rec = sm.tile([1, TC // NQ], F32, tag="recr", name="recr")
                    nc.vector.reciprocal(out=rec[:], in_=pav[32:33, fs])
                    rb = sm.tile([32, TC // NQ], F32, tag="rbl", name="rbl")
                    nc.gpsimd.partition_broadcast(rb[:], rec[:], channels=32)
                    rbs.append((fs, rb))
                for fs, rb in rbs:
                    nc.vector.tensor_mul(out=at2[:, 1, fs], in0=pav[0:32, fs],
                                         in1=rb[:])
                    emit_pair_proj(h, fs=fs, last=True)
                return
            rb = sm.tile([32, TC], F32, tag="rb", bufs=3, name="rb")
            rec = sm.tile([1, TC], F32, tag="rec", name="rec")
            nc.vector.reciprocal(out=rec[:], in_=pav[32:33, :])
            nc.gpsimd.partition_broadcast(rb[:], rec[:], channels=32)
            nc.vector.tensor_mul(out=at2[:, h % 2, :], in0=pav[0:32, :],
                                 in1=rb[:])
            if h % 2 == 1:
                emit_pair_proj(h)

        pavs = {}
        pend = None   # (pe_t, h, p) awaiting its AV matmul
        tail_q = []   # (head, global slot when its last AV was emitted)
        for g, (h, p) in enumerate(slot_seq):
            if g == 20:
                init_hout()
            oh, rh = h // 3, 32 * (h % 3)
            if h not in pavs:
                pavs[h] = ps_a.tile([33, TC], F32, tag="ps_a", name="ps_a")
            pss = [ps_s.tile([128, TC], F32, tag="ps_s", name="ps_s")
                   for _ in range(2)]
            for half in range(2):
                i = 2 * p + half
                nc.tensor.matmul(
                    pss[half][:],
                    k_sb[oh][rh:rh + 32, 128 * i:128 * (i + 1)],
                    q_sb[oh][rh:rh + 32, :],
                    start=True, stop=True)
            if pend is not None:
                pe_prev, hp, ppr = pend
                nc.tensor.matmul(
                    pavs[hp][:],
                    vt_sb[:, 2 * ppr:2 * ppr + 2, 33 * hp:33 * (hp + 1)],
                    pe_prev[:].rearrange("p (i t) -> p i t", i=2),
                    start=(ppr == 0), stop=(ppr == NPAIR - 1),
                    perf_mode=PM.DoubleRow)
                if ppr == NPAIR - 1:
                    tail_q.append((hp, g))
            if tail_q and g - tail_q[0][1] >= 18:
                th, _ = tail_q.pop(0)
                emit_head_tail(th, pavs.pop(th))
            pe_t = pex.tile([128, 2 * TC], F8, tag="pex", name="pex")
            # one exp instruction per 1-bank score tile: the 4-deep score
            # rotation decouples slot g+2's S matmuls from this slot's exp
            for half in range(2):
                cs = slice(TC * half, TC * (half + 1))
                if g in dve_exp_slots:
                    # Schraudolph exp: fused mult-add, saturating uint8
                    # convert; bytes are the fp8e4 encoding of ~e^(S-2)
                    nc.vector.tensor_scalar(
                        out=pe_t[:, cs].bitcast(U8), in0=pss[half][:],
                        scalar1=SCH_A, scalar2=SCH_B,
                        op0=ALU.mult, op1=ALU.add)
                else:
                    nc.scalar.activation(out=pe_t[:, cs], in_=pss[half][:],
                                         func=AF.Exp, bias=nbias[:])
            pend = (pe_t, h, p)
            for kind, arg in prod_for.get(g, ()):
                if kind == "v1":
                    emit_v_block(arg)
                else:
                    emit_k_chunk(*arg)
        for th, _ in tail_q:
            emit_head_tail(th, pavs.pop(th))
        pe_prev, hp, ppr = pend
        nc.tensor.matmul(
            pavs[hp][:],
            vt_sb[:, 2 * ppr:2 * ppr + 2, 33 * hp:33 * (hp + 1)],
            pe_prev[:].rearrange("p (i t) -> p i t", i=2),
            start=(ppr == 0), stop=(ppr == NPAIR - 1),
            perf_mode=PM.DoubleRow)
        emit_head_tail(hp, pavs.pop(hp), last=True)

    nc.compile()
    return nc


def host_prep(inputs):
    """Shared (core-independent) weight prep + per-core input maps."""
    import ml_dtypes

    x = np.ascontiguousarray(inputs["x"].reshape(C, T), dtype=np.float32)
    qkv_w = np.asarray(inputs["qkv_w"], dtype=np.float32)
    qkv_b = np.asarray(inputs["qkv_b"], dtype=np.float32)
    proj_w = np.asarray(inputs["proj_w"], dtype=np.float32)
    proj_b = np.asarray(inputs["proj_b"], dtype=np.float32)

    # heads laid out in 3 tiles of 128 rows at offsets {0,32,64}: head h ->
    # tile h//3, offset 32*(h%3)  (PE matmul base partition must be 0/32/64)
    def permute_qk(wT, b):                    # wT [C_in, 256], b [256]
        wp = np.zeros((C, 384), dtype=np.float32)
        bp = np.zeros((384, 1), dtype=np.float32)
        for h in range(NH):
            dst = 128 * (h // 3) + 32 * (h % 3)
            wp[:, dst:dst + 32] = wT[:, 32 * h:32 * h + 32]
            if b is not None:
                bp[dst:dst + 32, 0] = b[32 * h:32 * h + 32]
        return wp, bp

    # Exact GroupNorm fold (fp32, host): xn = A*x + B with A = gamma*istd,
    # B = beta - mean*A, per channel.  A scales the qkv weight columns; B
    # flows into the biases: q gets (bq + Wq@B)*SCALE2, k's bias is constant
    # along the softmax axis and cancels, v's goes into b_p via proj_w.
    gamma_c = np.asarray(inputs["gn_gamma"], np.float32)
    beta_c = np.asarray(inputs["gn_beta"], np.float32)
    xg = x.reshape(8, (C // 8) * T)
    mu_g = xg.mean(axis=1)
    istd_g = 1.0 / np.sqrt(xg.var(axis=1) + EPS)
    A = gamma_c * np.repeat(istd_g, C // 8)              # [C]
    B = beta_c - np.repeat(mu_g, C // 8) * A             # [C]

    wq_s = qkv_w[0:C] * A[None, :]
    wk_s = qkv_w[C:2 * C] * A[None, :]
    wv_s = qkv_w[2 * C:3 * C] * A[None, :]
    # SCALE2 (both attention scales) folded into Wq and bq on the host
    w_qT, b_qp = permute_qk(wq_s.T * SCALE2,
                            (qkv_b[0:C] + wq_s @ B) * SCALE2)
    w_kT, _ = permute_qk(wk_s.T, None)
    w_vT_n = wv_s.T                        # [C_in, C_v]
    w_vT = np.zeros((C, NH * 33), dtype=np.float32)
    for h in range(NH):
        w_vT[:, 33 * h:33 * h + 32] = w_vT_n[:, 32 * h:32 * h + 32]
    # fp8 qkv weights in DoubleRow pair layout [128, 2, 1040] (padded row)
    w_qkv = np.zeros((C, 1040), dtype=np.float32)
    w_qkv[:, 0:1032] = np.concatenate([w_qT, w_kT, w_vT], axis=1)
    w_8 = np.ascontiguousarray(
        w_qkv.reshape(2, 128, 1040).transpose(1, 0, 2)
    ).astype(ml_dtypes.float8_e4m3).reshape(128, 2 * 1040)
    # w_p32[c, h, o] = proj_w[o, 32h + c], as fp8e4 bytes
    w_p32 = np.ascontiguousarray(
        proj_w.reshape(C, NH, CHD).transpose(2, 1, 0)).reshape(CHD, NH * C)
    w_p8 = w_p32.astype(ml_dtypes.float8_e4m3)
    b_v = qkv_b[2 * C:3 * C] + wv_s @ B
    b_p = (proj_b + proj_w @ b_v).reshape(C, 1)

    gamma = gamma_c.reshape(2, 128).T
    beta = beta_c.reshape(2, 128).T
    bp2 = np.ascontiguousarray(b_p.reshape(2, 128).T)
    bq3 = b_qp.reshape(3, 128).T
    cvec = np.concatenate([gamma, beta, bp2, bq3], axis=1)  # [128, 9]

    # fp8 x (pair layout)
    x8 = x.astype(ml_dtypes.float8_e4m3)                 # [C, T]

    shared = {"x_8": x8, "w_p32": w_p8}
    cvec_u8 = np.ascontiguousarray(cvec).view(np.uint8)        # [128, 36]
    w8_u8 = w_8.view(np.uint8)                                 # [128, 2080]
    in_maps = []
    for cid in range(NCORES):
        m = dict(shared)
        ch = slice(TC * cid, TC * (cid + 1))
        m["x_c"] = np.ascontiguousarray(x[:, ch])
        x8c_u8 = np.ascontiguousarray(
            x8[:, ch].reshape(2, 128, TC).transpose(1, 0, 2)
        ).reshape(128, 2 * TC).view(np.uint8)
        blob = np.zeros((128, 3168), dtype=np.uint8)
        blob[:, 0:36] = cvec_u8
        blob[:, 64:64 + 2080] = w8_u8
        blob[:, 2144:2144 + 1024] = x8c_u8
        m["blob"] = blob
        in_maps.append(m)
    return in_maps


_NC_CACHE = None


def kernel(**inputs):
    global _NC_CACHE
    from concourse.bass_utils import run_bass_kernel_spmd

    if _NC_CACHE is None:
        _NC_CACHE = build_nc()
    in_maps = host_prep(inputs)
    res = run_bass_kernel_spmd(_NC_CACHE, in_maps, core_ids=list(range(NCORES)))
    outs = [np.asarray(r["out"]) for r in res.results]
    full = np.concatenate(outs, axis=1).reshape(1, C, 64, 64)
    return full.astype(np.float32)

